# revision 41
# baseline (speedup 1.0000x reference)
"""Trainium2 Bass kernel for nn_Attention_12000138625343.

Full multi-head attention layer (B=2, S=2048, E=1024, H=16, hd=64, interleaved
RoPE on q/k, non-causal softmax) run tensor-parallel over 8 NeuronCores:

  - heads sharded 2-per-core (w1 columns / qkv projection sharded),
  - x replicated, passed pre-transposed [E, B*S] so the contraction dim lands
    on SBUF partitions,
  - scores computed transposed [k, q]; the two heads' K=64 score matmuls are
    packed into disjoint PE row-groups (concurrent), one exp instruction
    covers both heads' [128, 1024] PSUM block,
  - attn@v accumulates rolling per k-chunk with a ones-column appended to v
    producing the softmax denominator; the divide runs entirely off the
    TensorEngine (DVE reciprocal + DRAM-bounce broadcast DMA + DVE multiply),
  - batch-1 qkv projection / batch-0 output projection matmul chains are
    dribbled into the attention k-chunk loop so the in-order PE stream never
    starves the exp pipeline for long,
  - two AllToAlls (one per batch) of the per-head attention output o^T
    convert head sharding into row sharding; the batch-0 A2A and its half of
    the w2 projection hide under batch-1 compute,
  - each core owns 256 rows of each batch; host reassembles.

Matmuls run in float32r (TF32-like, ~1e-4 relative error, full PE rate at
free-dim >= 256).
"""

import math

import numpy as np

import concourse.bass as bass
import concourse.mybir as mybir
import concourse.tile as tile
from concourse import bacc
from concourse.bass_utils import run_bass_kernel_spmd

B, S, E, H = 2, 2048, 1024, 16
HD = E // H  # 64
BASE = 10000.0
N_CORES = 8
HPC = H // N_CORES       # heads per core = 2
R = B * S                # 4096 flattened rows
RT = 512                 # rows per r-tile
NEC = E // 128           # 8 e-chunks of 128
QT = 512                 # q columns per q-tile
N_QT = S // QT           # 4 q-tiles per batch
KC = 128                 # k rows per k-chunk
N_KC = S // KC           # 16 k-chunks per batch
RPB = S // N_CORES       # rows per core per batch = 256

F32 = mybir.dt.float32
F32R = mybir.dt.float32r
EXPF = mybir.ActivationFunctionType.Exp

_COMPILED = {}


def _build_nc():
    nc = bacc.Bacc("TRN2", target_bir_lowering=False, debug=False,
                   num_devices=N_CORES)

    xT = nc.dram_tensor("xT", [E, R], F32, kind="ExternalInput").ap()
    wqT = nc.dram_tensor("wqT", [E, 128], F32, kind="ExternalInput").ap()
    wkT = nc.dram_tensor("wkT", [E, 128], F32, kind="ExternalInput").ap()
    wvT = nc.dram_tensor("wvT", [E, 128], F32, kind="ExternalInput").ap()
    w2T = nc.dram_tensor("w2T", [E, E], F32, kind="ExternalInput").ap()
    cosT = nc.dram_tensor("cosT", [128, S], F32, kind="ExternalInput").ap()
    sinT = nc.dram_tensor("sinT", [128, S], F32, kind="ExternalInput").ap()
    p2T = nc.dram_tensor("p2T", [128, 128], F32, kind="ExternalInput").ap()
    out = nc.dram_tensor("out", [2 * RPB, E], F32, kind="ExternalOutput").ap()

    with tile.TileContext(nc) as tc:
        _emit(tc, nc, xT, wqT, wkT, wvT, w2T, cosT, sinT, p2T, out)
    nc.compile()
    return nc


def _emit(tc, nc, xT, wqT, wkT, wvT, w2T, cosT, sinT, p2T, out):
    import contextlib
    ctx = contextlib.ExitStack()
    consts = ctx.enter_context(tc.tile_pool(name="consts", bufs=1))
    xtp = ctx.enter_context(tc.tile_pool(name="xtp", bufs=2))
    qkp = ctx.enter_context(tc.tile_pool(name="qkp", bufs=1))
    rawp = ctx.enter_context(tc.tile_pool(name="rawp", bufs=2))
    tmpp = ctx.enter_context(tc.tile_pool(name="tmpp", bufs=2))
    vp = ctx.enter_context(tc.tile_pool(name="vp", bufs=1))
    pp = ctx.enter_context(tc.tile_pool(name="pp", bufs=7))
    smallp = ctx.enter_context(tc.tile_pool(name="smallp", bufs=2))
    dramp = ctx.enter_context(tc.tile_pool(name="dramp", bufs=1, space="DRAM"))
    # PSUM budget (8 banks): qkv-shared 2 + sps 2 x 2 + av 2 = 8
    ps_qkv = ctx.enter_context(tc.tile_pool(name="ps_qkv", bufs=2, space="PSUM"))
    ps_sps = ctx.enter_context(tc.tile_pool(name="ps_sps", bufs=2, space="PSUM"))
    ps_av = ctx.enter_context(tc.tile_pool(name="ps_av", bufs=2, space="PSUM"))

    # ---- batched constant loads (single DMA each); q weights + first x tile
    # first so the first matmul can start ~10us in ----
    wq_f = xtp.tile([128, NEC, 128], F32, tag="recv", bufs=2, name="wq_f")
    nc.sync.dma_start(out=wq_f[:], in_=wqT.rearrange("(c p) f -> p c f", p=128))
    wq_all = consts.tile([128, NEC, 128], F32R, tag="wq", name="wq_all")
    nc.scalar.copy(wq_all[:], wq_f[:])
    xt0 = xtp.tile([128, NEC, RT], F32R, tag="xt", name="xt_0")
    nc.gpsimd.dma_start(
        out=xt0[:], in_=xT.rearrange("(c p) r -> p c r", p=128)[:, :, 0:RT])
    wk_all = consts.tile([128, NEC, 128], F32R, tag="wk", name="wk_all")
    nc.gpsimd.dma_start(out=wk_all[:], in_=wkT.rearrange("(c p) f -> p c f", p=128))
    wv_all = consts.tile([128, NEC, 128], F32R, tag="wv", name="wv_all")
    nc.gpsimd.dma_start(out=wv_all[:], in_=wvT.rearrange("(c p) f -> p c f", p=128))
    p2_sb = consts.tile([128, 128], F32R, tag="p2", name="p2_sb")
    nc.gpsimd.dma_start(out=p2_sb[:], in_=p2T[:, :])

    cos_sb = consts.tile([128, S], F32, tag="cos", name="cos_sb")
    nc.sync.dma_start(out=cos_sb[:], in_=cosT[:, :])
    sin_sb = consts.tile([128, S], F32, tag="sin", name="sin_sb")
    nc.sync.dma_start(out=sin_sb[:], in_=sinT[:, :])
    ones_f32 = consts.tile([128, 64], F32, tag="ones32", name="ones_f32")
    nc.vector.memset(ones_f32[:], 1.0)
    ones_r = consts.tile([1, 64], F32R, tag="onesr", name="ones_r")
    nc.vector.tensor_copy(ones_r[:], ones_f32[0:1, 0:64])

    # A2A buffers, one pair per (batch, q-tile): [8 chunks, 128 e-rows, 64 rows]
    # core j's chunk of q-tile qt carries s in [512qt + 64j, +64); each A2A
    # fires as soon as its q-tile's divide lands, so only the last 0.25 MB
    # collective is ever exposed at the kernel tail.
    send_d = {(b, qt): dramp.tile([N_CORES, 128, 64], F32, name=f"send{b}{qt}")
              for b in range(B) for qt in range(N_QT)}
    recv_d = {(b, qt): dramp.tile([N_CORES, 128, 64], F32, name=f"recv{b}{qt}")
              for b in range(B) for qt in range(N_QT)}

    qT_sb, kT_sb, v_sb = {}, {}, {}
    w2_sb = {}

    def emit_xt_load(rt):
        if rt == 0:
            return xt0
        t = xtp.tile([128, NEC, RT], F32R, tag="xt", name=f"xt_{rt}")
        nc.gpsimd.dma_start(
            out=t[:],
            in_=xT.rearrange("(c p) r -> p c r", p=128)[:, :, rt * RT:(rt + 1) * RT])
        return t

    def qkv_chains(rt, xt):
        """Return a list of closures, each emitting one matmul chain (+ its
        epilogue) for r-tile rt. Callers dribble these between attention
        steps to keep the in-order PE stream dense but never monolithic."""
        b, st = rt // N_QT, (rt % N_QT) * RT

        if b not in qT_sb:
            qT_sb[b] = qkp.tile([128, S], F32R, tag=f"qT{b}", name=f"qT{b}")
            kT_sb[b] = qkp.tile([128, S], F32R, tag=f"kT{b}", name=f"kT{b}")

        def qk_chain(kind, w_all, dst):
            state = {}
            def emit_a():
                acc = ps_qkv.tile([128, RT], F32, tag="qkv",
                                  name=f"{kind}acc{rt}")
                for ec in range(4):
                    nc.tensor.matmul(acc[:], w_all[:, ec, :], xt[:, ec, :],
                                     start=(ec == 0), stop=False)
                state["acc"] = acc
            def emit_b():
                acc = state.pop("acc")
                for ec in range(4, NEC):
                    nc.tensor.matmul(acc[:], w_all[:, ec, :], xt[:, ec, :],
                                     start=False, stop=(ec == NEC - 1))
                raw = rawp.tile([128, RT], F32R, tag="raw",
                                name=f"{kind}raw{rt}")
                nc.scalar.copy(raw[:], acc[:])
                rot = ps_qkv.tile([128, RT], F32, tag="qkv",
                                  name=f"{kind}rot{rt}")
                nc.tensor.matmul(rot[:], p2_sb[:], raw[:], start=True, stop=True)
                t1 = tmpp.tile([128, RT], F32, tag="ropet", name=f"{kind}t1_{rt}")
                nc.vector.tensor_mul(t1[:], raw[:].bitcast(F32),
                                     cos_sb[:, st:st + RT])
                t2 = tmpp.tile([128, RT], F32, tag="ropet", name=f"{kind}t2_{rt}")
                nc.vector.tensor_mul(t2[:], rot[:], sin_sb[:, st:st + RT])
                nc.vector.tensor_add(dst[:, st:st + RT], t1[:], t2[:])
            return [emit_a, emit_b]

        def v_chain(sub):
            def emit():
                vacc = ps_qkv.tile([128, 128], F32, tag="qkv",
                                   name=f"vacc{rt}_{sub}")
                for ec in range(NEC):
                    nc.tensor.matmul(vacc[:],
                                     xt[:, ec, sub * 128:(sub + 1) * 128],
                                     wv_all[:, ec, :],
                                     start=(ec == 0), stop=(ec == NEC - 1))
                kc = (rt % N_QT) * 4 + sub
                for h in range(HPC):
                    vt = vp.tile([128, 65], F32R, tag=f"v{b}{h}{kc}",
                                 name=f"v{b}{h}{kc}")
                    nc.vector.tensor_copy(vt[:, 0:64],
                                          vacc[:, h * 64:(h + 1) * 64])
                    nc.vector.tensor_copy(vt[:, 64:65], ones_f32[:, 0:1])
                    v_sb[(b, h, kc)] = vt
            return emit

        return qk_chain("q", wq_all, qT_sb[b]) + \
               qk_chain("k", wk_all, kT_sb[b]) + \
               [v_chain(sub) for sub in range(4)]

    def proj_chains(b, rb):
        """Output projection for my 128 rows of (batch b, q-tile pair rb):
        64 rows from q-tile 2rb + 64 rows from q-tile 2rb+1, loaded side by
        side so the matmul keeps a full 128-row stationary operand. The recv
        load is emitted lazily by the first chain so that building the chain
        list never precedes the collectives' emission."""
        state0 = {}
        def get_recv():
            if "t" not in state0:
                t = xtp.tile([128, NEC, 128], F32R, tag="recv", bufs=2,
                             name=f"recv{b}{rb}")
                nc.gpsimd.dma_start(
                    out=t[:, :, 0:64],
                    in_=recv_d[(b, 2 * rb)].rearrange("c p r -> p c r"))
                nc.gpsimd.dma_start(
                    out=t[:, :, 64:128],
                    in_=recv_d[(b, 2 * rb + 1)].rearrange("c p r -> p c r"))
                state0["t"] = t
            return state0["t"]
        chains = []
        for rblk in [rb]:
            for ft in range(2):
                state = {}
                def emit_a(rblk=rblk, ft=ft, state=state):
                    recv_sb = get_recv()
                    # qkv psum tag: free during attention (projection is done)
                    ops = ps_qkv.tile([128, 512], F32, tag="qkv",
                                      name=f"ops{b}_{rblk}_{ft}")
                    for ec in range(4):
                        nc.tensor.matmul(
                            ops[:],
                            recv_sb[:, ec, :],
                            w2_sb[0][:, ec, ft * 512:(ft + 1) * 512],
                            start=(ec == 0), stop=False)
                    state["ops"] = ops
                def emit_b(rblk=rblk, ft=ft, state=state):
                    recv_sb = get_recv()
                    ops = state.pop("ops")
                    for ec in range(4, NEC):
                        nc.tensor.matmul(
                            ops[:],
                            recv_sb[:, ec, :],
                            w2_sb[0][:, ec, ft * 512:(ft + 1) * 512],
                            start=False, stop=(ec == NEC - 1))
                    ot = tmpp.tile([128, 512], F32, tag="ropet",
                                   name=f"ot{b}_{rblk}_{ft}")
                    nc.scalar.copy(ot[:], ops[:])
                    # out rows: [b0rb0, b0rb1, b1rb0, b1rb1] blocks of 128
                    ob = 2 * b + rblk
                    nc.sync.dma_start(
                        out=out[ob * 128:(ob + 1) * 128,
                                ft * 512:(ft + 1) * 512],
                        in_=ot[:])
                chains.append(emit_a)
                chains.append(emit_b)
        return chains

    def emit_divide(b, qt, avs):
        """Divide by the softmax denominator (row 64 of av) and stage into
        the A2A send buffer. PE-free: broadcast via a DRAM bounce DMA.
        All DMAs on HWDGE queues (sync/scalar) so the collective sitting on
        the gpsimd queue can never block them."""
        last = (b == B - 1 and qt == N_QT - 1)
        for h in range(HPC):
            # evict the accumulator to SBUF immediately: releases the PSUM
            # slot so the next q-tile's attn@v never waits on this divide
            oraw = smallp.tile([65, QT], F32, tag="oraw", name=f"oraw{b}{h}{qt}")
            nc.scalar.copy(oraw[:], avs[h][:])
            rcp = smallp.tile([1, QT], F32R, tag="rcp", name=f"rcp{b}{h}{qt}")
            with nc.allow_low_precision(reason="f32r reciprocal ~1e-4"):
                nc.vector.reciprocal(rcp[:], oraw[64:65, :])
            bcs = smallp.tile([64, QT], F32, tag="bcs", name=f"bcs{b}{h}{qt}")
            if last:
                # PE is idle at the kernel tail: broadcast via a K=1 matmul
                # instead of the DRAM-bounce DMA round trip
                bcq = ps_av.tile([64, QT], F32, tag="av", name=f"bcq{b}{h}{qt}")
                nc.tensor.matmul(bcq[:], ones_r[:], rcp[:], start=True,
                                 stop=True)
                nc.scalar.copy(bcs[:], bcq[:])
            else:
                rcp_d = dramp.tile([1, QT], F32, tag="rcpd", bufs=4,
                                   name=f"rcpd{b}{h}{qt}")
                nc.sync.dma_start(out=rcp_d[:], in_=rcp[:].bitcast(F32))
                bcast = bass.AP(tensor=rcp_d.tensor, offset=rcp_d.offset,
                                ap=[[0, 64]] + list(rcp_d.ap[1:]))
                nc.sync.dma_start(out=bcs[:], in_=bcast)
            odiv = smallp.tile([64, QT], F32, tag="odiv", name=f"odiv{b}{h}{qt}")
            nc.vector.tensor_mul(odiv[:], oraw[0:64, :], bcs[:])
            for j in range(N_CORES):
                nc.sync.dma_start(
                    out=send_d[(b, qt)][j, h * 64:(h + 1) * 64, :],
                    in_=odiv[:, j * 64:(j + 1) * 64])

    def emit_attention_batch(b, dribble):
        """All 4 q-tiles of a batch as one rolling pipeline over 64+LAG
        (qt, kc) units: scores+exp lead, attn@v trails by LAG units, the
        divide chain fires as each q-tile's accumulation completes.  One
        dribble chain (qkv projection / output projection) is popped every
        other unit to keep the in-order PE stream dense."""
        scale = 1.0 / math.sqrt(HD)
        NU = N_QT * N_KC
        LAG = 5
        pts = {}
        avs = {}
        for u in range(NU + LAG):
            if u < NU:
                qt, kc = divmod(u, N_KC)
                if kc == 0:
                    avs[qt] = [ps_av.tile([65, QT], F32, tag="av",
                                          name=f"av{b}{h}{qt}")
                               for h in range(HPC)]
                sps = ps_sps.tile([128, 2 * QT], F32, tag="sps",
                                  name=f"s{b}{qt}_{kc}")
                for h in range(HPC):
                    hof = h * 64
                    nc.tensor.matmul(
                        sps[:, h * QT:(h + 1) * QT],
                        kT_sb[b][hof:hof + 64, kc * KC:(kc + 1) * KC],
                        qT_sb[b][hof:hof + 64, qt * QT:(qt + 1) * QT],
                        start=True, stop=True)
                pt = pp.tile([128, 2 * QT], F32R, tag="p", name=f"p{b}{qt}_{kc}")
                nc.scalar.activation(pt[:], sps[:], EXPF, scale=scale)
                pts[u] = pt
            if u >= LAG:
                j = u - LAG
                qt2, kc2 = divmod(j, N_KC)
                for h in range(HPC):
                    nc.tensor.matmul(avs[qt2][h][:], v_sb[(b, h, kc2)][:],
                                     pts[j][:, h * QT:(h + 1) * QT],
                                     start=(kc2 == 0), stop=(kc2 == N_KC - 1))
                del pts[j]
                if kc2 == N_KC - 1:
                    emit_divide(b, qt2, avs.pop(qt2))
                    if qt2 < N_QT - 1:
                        emit_a2a(b, qt2)
            # one chain per two units, ramping up near the end so no
            # backlog remains to run as a monolithic lump afterwards
            if dribble and dribble[0][0] <= u and (
                    u % 2 == 1 or 2 * len(dribble) >= (NU + LAG - u)):
                dribble.pop(0)[1]()

    def emit_a2a(b, qt):
        nc.gpsimd.collective_compute(
            "AllToAll", mybir.AluOpType.bypass,
            replica_groups=[list(range(N_CORES))],
            ins=[send_d[(b, qt)].opt()], outs=[recv_d[(b, qt)].opt()])

    # ---------------- emission ----------------
    for rt in range(N_QT):             # batch-0 projection: pure PE stretch
        xt = emit_xt_load(rt)
        for chain in qkv_chains(rt, xt):
            chain()
    # warm the collective path (cold-start ~8us); emitted here so the wait on
    # the gpsimd queue never delays the critical first x/weight loads
    cwu_s = dramp.tile([N_CORES, 8], F32, tag="cwus", name="cwu_s")
    cwu_r = dramp.tile([N_CORES, 8], F32, tag="cwur", name="cwu_r")
    nc.sync.dma_start(out=cwu_s.rearrange("c r -> (c r)")[None, :],
                      in_=ones_f32[0:1, 0:64])
    nc.gpsimd.collective_compute(
        "AllToAll", mybir.AluOpType.bypass,
        replica_groups=[list(range(N_CORES))],
        ins=[cwu_s.opt()], outs=[cwu_r.opt()])

    # w2 load early: 4 MB, overlaps the batch-0 attention stretch
    w2_sb[0] = consts.tile([128, NEC, E], F32R, tag="w2", name="w2_all")
    nc.gpsimd.dma_start(out=w2_sb[0][:],
                        in_=w2T.rearrange("(c p) f -> p c f", p=128))
    # batch-0 attention with batch-1 qkv dribbled in
    dribble = []
    for qt in range(N_QT):
        rt = N_QT + qt
        xt = emit_xt_load(rt)
        dribble.extend(qkv_chains(rt, xt))
    dribble = [(1, c) for c in dribble]
    emit_attention_batch(0, dribble)
    for _, chain in dribble:
        chain()
    del dribble[:]
    emit_a2a(0, N_QT - 1)              # last quarter, fires at batch-0 end

    # both batch-0 halves complete early in batch-1 attention;
    # batch-1 half 0's A2A fires mid-batch, its projection runs at the tail
    dribble = [(24, c) for c in proj_chains(0, 0) + proj_chains(0, 1)]
    dribble += [(58, c) for c in proj_chains(1, 0)]
    emit_attention_batch(1, dribble)
    for _, chain in dribble:
        chain()
    emit_a2a(1, N_QT - 1)
    for chain in proj_chains(1, 1):
        chain()
    ctx.close()


def _host_prep(x, w1, w2):
    x = np.ascontiguousarray(np.asarray(x, dtype=np.float32))
    w1 = np.ascontiguousarray(np.asarray(w1, dtype=np.float32))
    w2 = np.ascontiguousarray(np.asarray(w2, dtype=np.float32))

    xT = np.ascontiguousarray(x.reshape(R, E).T)          # [E, R]
    w2T = np.ascontiguousarray(w2.T)                      # [E, E]

    theta = 1.0 / (BASE ** (np.arange(0, HD, 2, dtype=np.float32) / HD))
    enc = np.arange(S, dtype=np.float32)[:, None] * theta[None, :]
    enc = np.repeat(enc, 2, axis=-1)                      # [s, 64]
    cos1 = np.cos(enc).T.astype(np.float32)               # [64, S]
    sin1 = np.sin(enc).T.astype(np.float32)
    cosT = np.ascontiguousarray(np.concatenate([cos1, cos1], axis=0))
    sinT = np.ascontiguousarray(np.concatenate([sin1, sin1], axis=0))

    m64 = np.zeros((HD, HD), dtype=np.float32)
    for i in range(HD // 2):
        m64[2 * i, 2 * i + 1] = -1.0
        m64[2 * i + 1, 2 * i] = 1.0
    m128 = np.zeros((128, 128), dtype=np.float32)
    m128[:64, :64] = m64
    m128[64:, 64:] = m64
    p2T = np.ascontiguousarray(m128.T)

    in_maps = []
    for c in range(N_CORES):
        hA, hB = HPC * c, HPC * c + 1
        def rows(base):
            return np.concatenate(
                [w1[base + hA * HD: base + (hA + 1) * HD, :],
                 w1[base + hB * HD: base + (hB + 1) * HD, :]], axis=0)
        in_maps.append({
            "xT": xT,
            "wqT": np.ascontiguousarray(rows(0).T),
            "wkT": np.ascontiguousarray(rows(E).T),
            "wvT": np.ascontiguousarray(rows(2 * E).T),
            "w2T": w2T,
            "cosT": cosT,
            "sinT": sinT,
            "p2T": p2T,
        })
    return in_maps


def kernel(x, w1, w2, _trace=False):
    if "nc" not in _COMPILED:
        _COMPILED["nc"] = _build_nc()
    nc = _COMPILED["nc"]
    in_maps = _host_prep(x, w1, w2)
    res = run_bass_kernel_spmd(nc, in_maps, core_ids=list(range(N_CORES)),
                               trace=_trace)
    _COMPILED["last_result"] = res
    # core c returns [512, E] as four 128-row blocks (b, rb), each holding
    # 64 rows of q-tile 2rb (s = 512*2rb + 64c ..) then 64 rows of 2rb+1.
    full = np.empty((B, S, E), dtype=np.float32)
    for c in range(N_CORES):
        blk = res.results[c]["out"]
        for b in range(B):
            for rb in range(2):
                base = 128 * (2 * b + rb)
                for t in range(2):
                    qt = 2 * rb + t
                    s0 = 512 * qt + 64 * c
                    full[b, s0:s0 + 64] = blk[base + 64 * t:base + 64 * (t + 1)]
    return full


# revision 42
# speedup vs baseline: 1.1026x; 1.1026x over previous
"""Trainium2 Bass kernel for nn_Attention_12000138625343.

Full multi-head attention layer (B=2, S=2048, E=1024, H=16, hd=64, interleaved
RoPE on q/k, non-causal softmax) run tensor-parallel over 8 NeuronCores:

  - heads sharded 2-per-core (w1 columns / qkv projection sharded),
  - x replicated, passed pre-transposed [E, B*S] so the contraction dim lands
    on SBUF partitions,
  - scores computed transposed [k, q]; the two heads' K=64 score matmuls are
    packed into disjoint PE row-groups (concurrent), one exp instruction
    covers both heads' [128, 1024] PSUM block,
  - attn@v accumulates rolling per k-chunk with a ones-column appended to v
    producing the softmax denominator; the divide runs entirely off the
    TensorEngine (DVE reciprocal + DRAM-bounce broadcast DMA + DVE multiply),
  - batch-1 qkv projection / batch-0 output projection matmul chains are
    dribbled into the attention k-chunk loop so the in-order PE stream never
    starves the exp pipeline for long,
  - two AllToAlls (one per batch) of the per-head attention output o^T
    convert head sharding into row sharding; the batch-0 A2A and its half of
    the w2 projection hide under batch-1 compute,
  - each core owns 256 rows of each batch; host reassembles.

Matmuls run in float32r (TF32-like, ~1e-4 relative error, full PE rate at
free-dim >= 256).
"""

import math

import numpy as np

import concourse.bass as bass
import concourse.mybir as mybir
import concourse.tile as tile
from concourse import bacc
from concourse.bass_utils import run_bass_kernel_spmd

B, S, E, H = 2, 2048, 1024, 16
HD = E // H  # 64
BASE = 10000.0
N_CORES = 8
HPC = H // N_CORES       # heads per core = 2
R = B * S                # 4096 flattened rows
RT = 512                 # rows per r-tile
NEC = E // 128           # 8 e-chunks of 128
QT = 512                 # q columns per q-tile
N_QT = S // QT           # 4 q-tiles per batch
KC = 128                 # k rows per k-chunk
N_KC = S // KC           # 16 k-chunks per batch
RPB = S // N_CORES       # rows per core per batch = 256

F32 = mybir.dt.float32
F32R = mybir.dt.float32r
EXPF = mybir.ActivationFunctionType.Exp

_COMPILED = {}


def _build_nc():
    nc = bacc.Bacc("TRN2", target_bir_lowering=False, debug=False,
                   num_devices=N_CORES)

    xT = nc.dram_tensor("xT", [E, R], F32, kind="ExternalInput").ap()
    wqT = nc.dram_tensor("wqT", [E, 128], F32, kind="ExternalInput").ap()
    wkT = nc.dram_tensor("wkT", [E, 128], F32, kind="ExternalInput").ap()
    wvT = nc.dram_tensor("wvT", [E, 128], F32, kind="ExternalInput").ap()
    w2T = nc.dram_tensor("w2T", [E, E], F32, kind="ExternalInput").ap()
    cosT = nc.dram_tensor("cosT", [128, S], F32, kind="ExternalInput").ap()
    sinT = nc.dram_tensor("sinT", [128, S], F32, kind="ExternalInput").ap()
    p2T = nc.dram_tensor("p2T", [128, 128], F32, kind="ExternalInput").ap()
    out = nc.dram_tensor("out", [2 * RPB, E], F32, kind="ExternalOutput").ap()

    with tile.TileContext(nc) as tc:
        _emit(tc, nc, xT, wqT, wkT, wvT, w2T, cosT, sinT, p2T, out)
    nc.compile()
    return nc


def _emit(tc, nc, xT, wqT, wkT, wvT, w2T, cosT, sinT, p2T, out):
    import contextlib
    ctx = contextlib.ExitStack()
    consts = ctx.enter_context(tc.tile_pool(name="consts", bufs=1))
    xtp = ctx.enter_context(tc.tile_pool(name="xtp", bufs=2))
    qkp = ctx.enter_context(tc.tile_pool(name="qkp", bufs=1))
    rawp = ctx.enter_context(tc.tile_pool(name="rawp", bufs=2))
    tmpp = ctx.enter_context(tc.tile_pool(name="tmpp", bufs=2))
    vp = ctx.enter_context(tc.tile_pool(name="vp", bufs=1))
    pp = ctx.enter_context(tc.tile_pool(name="pp", bufs=7))
    smallp = ctx.enter_context(tc.tile_pool(name="smallp", bufs=2))
    dramp = ctx.enter_context(tc.tile_pool(name="dramp", bufs=1, space="DRAM"))
    # PSUM budget (8 banks): qkv-shared 2 + sps 2 x 2 + av 2 = 8
    ps_qkv = ctx.enter_context(tc.tile_pool(name="ps_qkv", bufs=2, space="PSUM"))
    ps_sps = ctx.enter_context(tc.tile_pool(name="ps_sps", bufs=2, space="PSUM"))
    ps_av = ctx.enter_context(tc.tile_pool(name="ps_av", bufs=2, space="PSUM"))

    # ---- batched constant loads (single DMA each); q weights + first x tile
    # first so the first matmul can start ~10us in ----
    wq_f = xtp.tile([128, NEC, 128], F32, tag="recv", bufs=2, name="wq_f")
    nc.sync.dma_start(out=wq_f[:], in_=wqT.rearrange("(c p) f -> p c f", p=128))
    wq_all = consts.tile([128, NEC, 128], F32R, tag="wq", name="wq_all")
    nc.scalar.copy(wq_all[:], wq_f[:])
    xt0 = xtp.tile([128, NEC, RT], F32R, tag="xt", name="xt_0")
    nc.gpsimd.dma_start(
        out=xt0[:], in_=xT.rearrange("(c p) r -> p c r", p=128)[:, :, 0:RT])
    wk_all = consts.tile([128, NEC, 128], F32R, tag="wk", name="wk_all")
    nc.gpsimd.dma_start(out=wk_all[:], in_=wkT.rearrange("(c p) f -> p c f", p=128))
    wv_all = consts.tile([128, NEC, 128], F32R, tag="wv", name="wv_all")
    nc.gpsimd.dma_start(out=wv_all[:], in_=wvT.rearrange("(c p) f -> p c f", p=128))
    p2_sb = consts.tile([128, 128], F32R, tag="p2", name="p2_sb")
    nc.gpsimd.dma_start(out=p2_sb[:], in_=p2T[:, :])

    cos_sb = consts.tile([128, S], F32, tag="cos", name="cos_sb")
    nc.sync.dma_start(out=cos_sb[:], in_=cosT[:, :])
    sin_sb = consts.tile([128, S], F32, tag="sin", name="sin_sb")
    nc.sync.dma_start(out=sin_sb[:], in_=sinT[:, :])
    ones_f32 = consts.tile([128, 64], F32, tag="ones32", name="ones_f32")
    nc.vector.memset(ones_f32[:], 1.0)
    ones_r = consts.tile([1, 64], F32R, tag="onesr", name="ones_r")
    nc.vector.tensor_copy(ones_r[:], ones_f32[0:1, 0:64])

    # A2A buffers, one pair per (batch, half): [8 chunks, 128 e-rows, 128 rows]
    # half 0 carries s in [128j, 128j+128) (ready after q-tile 1),
    # half 1 carries s in [1024+128j, ...) (ready after q-tile 3).
    send_d = {(b, hf): dramp.tile([N_CORES, 128, 128], F32, name=f"send{b}{hf}")
              for b in range(B) for hf in range(2)}
    recv_d = {(b, hf): dramp.tile([N_CORES, 128, 128], F32, name=f"recv{b}{hf}")
              for b in range(B) for hf in range(2)}

    qT_sb, kT_sb, v_sb = {}, {}, {}
    w2_sb = {}

    def emit_xt_load(rt):
        if rt == 0:
            return xt0
        t = xtp.tile([128, NEC, RT], F32R, tag="xt", name=f"xt_{rt}")
        nc.gpsimd.dma_start(
            out=t[:],
            in_=xT.rearrange("(c p) r -> p c r", p=128)[:, :, rt * RT:(rt + 1) * RT])
        return t

    def qkv_chains(rt, xt):
        """Return a list of closures, each emitting one matmul chain (+ its
        epilogue) for r-tile rt. Callers dribble these between attention
        steps to keep the in-order PE stream dense but never monolithic."""
        b, st = rt // N_QT, (rt % N_QT) * RT

        if b not in qT_sb:
            qT_sb[b] = qkp.tile([128, S], F32R, tag=f"qT{b}", name=f"qT{b}")
            kT_sb[b] = qkp.tile([128, S], F32R, tag=f"kT{b}", name=f"kT{b}")

        def qk_chain(kind, w_all, dst):
            state = {}
            def emit_a():
                acc = ps_qkv.tile([128, RT], F32, tag="qkv",
                                  name=f"{kind}acc{rt}")
                for ec in range(4):
                    nc.tensor.matmul(acc[:], w_all[:, ec, :], xt[:, ec, :],
                                     start=(ec == 0), stop=False)
                state["acc"] = acc
            def emit_b():
                acc = state.pop("acc")
                for ec in range(4, NEC):
                    nc.tensor.matmul(acc[:], w_all[:, ec, :], xt[:, ec, :],
                                     start=False, stop=(ec == NEC - 1))
                raw = rawp.tile([128, RT], F32R, tag="raw",
                                name=f"{kind}raw{rt}")
                nc.scalar.copy(raw[:], acc[:])
                rot = ps_qkv.tile([128, RT], F32, tag="qkv",
                                  name=f"{kind}rot{rt}")
                nc.tensor.matmul(rot[:], p2_sb[:], raw[:], start=True, stop=True)
                t1 = tmpp.tile([128, RT], F32, tag="ropet", name=f"{kind}t1_{rt}")
                nc.vector.tensor_mul(t1[:], raw[:].bitcast(F32),
                                     cos_sb[:, st:st + RT])
                t2 = tmpp.tile([128, RT], F32, tag="ropet", name=f"{kind}t2_{rt}")
                nc.vector.tensor_mul(t2[:], rot[:], sin_sb[:, st:st + RT])
                nc.vector.tensor_add(dst[:, st:st + RT], t1[:], t2[:])
            return [emit_a, emit_b]

        def v_chain(sub):
            def emit():
                vacc = ps_qkv.tile([128, 128], F32, tag="qkv",
                                   name=f"vacc{rt}_{sub}")
                for ec in range(NEC):
                    nc.tensor.matmul(vacc[:],
                                     xt[:, ec, sub * 128:(sub + 1) * 128],
                                     wv_all[:, ec, :],
                                     start=(ec == 0), stop=(ec == NEC - 1))
                kc = (rt % N_QT) * 4 + sub
                for h in range(HPC):
                    vt = vp.tile([128, 65], F32R, tag=f"v{b}{h}{kc}",
                                 name=f"v{b}{h}{kc}")
                    nc.vector.tensor_copy(vt[:, 0:64],
                                          vacc[:, h * 64:(h + 1) * 64])
                    nc.vector.tensor_copy(vt[:, 64:65], ones_f32[:, 0:1])
                    v_sb[(b, h, kc)] = vt
            return emit

        return qk_chain("q", wq_all, qT_sb[b]) + \
               qk_chain("k", wk_all, kT_sb[b]) + \
               [v_chain(sub) for sub in range(4)]

    def proj_chains(b, hf):
        """Output projection for my 128 rows of (batch b, half hf).
        The recv load is emitted lazily by the first chain so that building
        the chain list never precedes the collective's emission."""
        state0 = {}
        def get_recv():
            if "t" not in state0:
                t = xtp.tile([128, NEC, 128], F32R, tag="recv", bufs=2,
                             name=f"recv{b}{hf}")
                nc.gpsimd.dma_start(
                    out=t[:], in_=recv_d[(b, hf)].rearrange("c p r -> p c r"))
                state0["t"] = t
            return state0["t"]
        chains = []
        for rblk in [hf]:
            for ft in range(2):
                state = {}
                def emit_a(rblk=rblk, ft=ft, state=state):
                    recv_sb = get_recv()
                    # qkv psum tag: free during attention (projection is done)
                    ops = ps_qkv.tile([128, 512], F32, tag="qkv",
                                      name=f"ops{b}_{rblk}_{ft}")
                    for ec in range(4):
                        nc.tensor.matmul(
                            ops[:],
                            recv_sb[:, ec, :],
                            w2_sb[0][:, ec, ft * 512:(ft + 1) * 512],
                            start=(ec == 0), stop=False)
                    state["ops"] = ops
                def emit_b(rblk=rblk, ft=ft, state=state):
                    recv_sb = get_recv()
                    ops = state.pop("ops")
                    for ec in range(4, NEC):
                        nc.tensor.matmul(
                            ops[:],
                            recv_sb[:, ec, :],
                            w2_sb[0][:, ec, ft * 512:(ft + 1) * 512],
                            start=False, stop=(ec == NEC - 1))
                    ot = tmpp.tile([128, 512], F32, tag="ropet",
                                   name=f"ot{b}_{rblk}_{ft}")
                    nc.scalar.copy(ot[:], ops[:])
                    # out rows: [b0h0, b0h1, b1h0, b1h1] blocks of 128
                    ob = 2 * b + rblk
                    nc.sync.dma_start(
                        out=out[ob * 128:(ob + 1) * 128,
                                ft * 512:(ft + 1) * 512],
                        in_=ot[:])
                chains.append(emit_a)
                chains.append(emit_b)
        return chains

    def emit_divide(b, qt, avs):
        """Divide by the softmax denominator (row 64 of av) and stage into
        the A2A send buffer. PE-free: broadcast via a DRAM bounce DMA.
        All DMAs on HWDGE queues (sync/scalar) so the collective sitting on
        the gpsimd queue can never block them."""
        last = (b == B - 1 and qt == N_QT - 1)
        for h in range(HPC):
            # evict the accumulator to SBUF immediately: releases the PSUM
            # slot so the next q-tile's attn@v never waits on this divide
            oraw = smallp.tile([65, QT], F32, tag="oraw", name=f"oraw{b}{h}{qt}")
            nc.scalar.copy(oraw[:], avs[h][:])
            rcp = smallp.tile([1, QT], F32R, tag="rcp", name=f"rcp{b}{h}{qt}")
            with nc.allow_low_precision(reason="f32r reciprocal ~1e-4"):
                nc.vector.reciprocal(rcp[:], oraw[64:65, :])
            bcs = smallp.tile([64, QT], F32, tag="bcs", name=f"bcs{b}{h}{qt}")
            if last:
                # PE is idle at the kernel tail: broadcast via a K=1 matmul
                # instead of the DRAM-bounce DMA round trip
                bcq = ps_av.tile([64, QT], F32, tag="av", name=f"bcq{b}{h}{qt}")
                nc.tensor.matmul(bcq[:], ones_r[:], rcp[:], start=True,
                                 stop=True)
                nc.scalar.copy(bcs[:], bcq[:])
            else:
                rcp_d = dramp.tile([1, QT], F32, tag="rcpd", bufs=4,
                                   name=f"rcpd{b}{h}{qt}")
                nc.sync.dma_start(out=rcp_d[:], in_=rcp[:].bitcast(F32))
                bcast = bass.AP(tensor=rcp_d.tensor, offset=rcp_d.offset,
                                ap=[[0, 64]] + list(rcp_d.ap[1:]))
                nc.sync.dma_start(out=bcs[:], in_=bcast)
            odiv = smallp.tile([64, QT], F32, tag="odiv", name=f"odiv{b}{h}{qt}")
            nc.vector.tensor_mul(odiv[:], oraw[0:64, :], bcs[:])
            # q-tile qt covers s in [512qt, 512qt+512): half hf = qt // 2,
            # destination cores j = 4*(qt%2) .. +4, 128 columns each
            hf = qt // 2
            for jj in range(4):
                j = 4 * (qt % 2) + jj
                nc.sync.dma_start(
                    out=send_d[(b, hf)][j, h * 64:(h + 1) * 64, :],
                    in_=odiv[:, jj * 128:(jj + 1) * 128])

    def emit_attention_batch(b, dribble):
        """All 4 q-tiles of a batch as one rolling pipeline over 64+LAG
        (qt, kc) units: scores+exp lead, attn@v trails by LAG units, the
        divide chain fires as each q-tile's accumulation completes.  One
        dribble chain (qkv projection / output projection) is popped every
        other unit to keep the in-order PE stream dense."""
        scale = 1.0 / math.sqrt(HD)
        NU = N_QT * N_KC
        LAG = 5
        pts = {}
        avs = {}
        for u in range(NU + LAG):
            if u < NU:
                qt, kc = divmod(u, N_KC)
                if kc == 0:
                    avs[qt] = [ps_av.tile([65, QT], F32, tag="av",
                                          name=f"av{b}{h}{qt}")
                               for h in range(HPC)]
                sps = ps_sps.tile([128, 2 * QT], F32, tag="sps",
                                  name=f"s{b}{qt}_{kc}")
                for h in range(HPC):
                    hof = h * 64
                    nc.tensor.matmul(
                        sps[:, h * QT:(h + 1) * QT],
                        kT_sb[b][hof:hof + 64, kc * KC:(kc + 1) * KC],
                        qT_sb[b][hof:hof + 64, qt * QT:(qt + 1) * QT],
                        start=True, stop=True)
                pt = pp.tile([128, 2 * QT], F32R, tag="p", name=f"p{b}{qt}_{kc}")
                nc.scalar.activation(pt[:], sps[:], EXPF, scale=scale)
                pts[u] = pt
            if u >= LAG:
                j = u - LAG
                qt2, kc2 = divmod(j, N_KC)
                for h in range(HPC):
                    nc.tensor.matmul(avs[qt2][h][:], v_sb[(b, h, kc2)][:],
                                     pts[j][:, h * QT:(h + 1) * QT],
                                     start=(kc2 == 0), stop=(kc2 == N_KC - 1))
                del pts[j]
                if kc2 == N_KC - 1:
                    emit_divide(b, qt2, avs.pop(qt2))
                    if qt2 == 1:
                        emit_a2a(b, 0)
            # one chain per two units, ramping up near the end so no
            # backlog remains to run as a monolithic lump afterwards
            if dribble and dribble[0][0] <= u and (
                    u % 2 == 1 or 2 * len(dribble) >= (NU + LAG - u)):
                dribble.pop(0)[1]()

    def emit_a2a(b, hf):
        nc.gpsimd.collective_compute(
            "AllToAll", mybir.AluOpType.bypass,
            replica_groups=[list(range(N_CORES))],
            ins=[send_d[(b, hf)].opt()], outs=[recv_d[(b, hf)].opt()])

    # ---------------- emission ----------------
    for rt in range(N_QT):             # batch-0 projection: pure PE stretch
        xt = emit_xt_load(rt)
        for chain in qkv_chains(rt, xt):
            chain()
    # warm the collective path (cold-start ~8us); emitted here so the wait on
    # the gpsimd queue never delays the critical first x/weight loads
    cwu_s = dramp.tile([N_CORES, 8], F32, tag="cwus", name="cwu_s")
    cwu_r = dramp.tile([N_CORES, 8], F32, tag="cwur", name="cwu_r")
    nc.sync.dma_start(out=cwu_s.rearrange("c r -> (c r)")[None, :],
                      in_=ones_f32[0:1, 0:64])
    nc.gpsimd.collective_compute(
        "AllToAll", mybir.AluOpType.bypass,
        replica_groups=[list(range(N_CORES))],
        ins=[cwu_s.opt()], outs=[cwu_r.opt()])

    # w2 load early: 4 MB, overlaps the batch-0 attention stretch
    w2_sb[0] = consts.tile([128, NEC, E], F32R, tag="w2", name="w2_all")
    nc.gpsimd.dma_start(out=w2_sb[0][:],
                        in_=w2T.rearrange("(c p) f -> p c f", p=128))
    # batch-0 attention with batch-1 qkv dribbled in
    dribble = []
    for qt in range(N_QT):
        rt = N_QT + qt
        xt = emit_xt_load(rt)
        dribble.extend(qkv_chains(rt, xt))
    dribble = [(1, c) for c in dribble]
    emit_attention_batch(0, dribble)
    for _, chain in dribble:
        chain()
    del dribble[:]
    emit_a2a(0, 1)                     # second half, fires at batch-0 end

    # both batch-0 halves complete early in batch-1 attention;
    # batch-1 half 0's A2A fires mid-batch, its projection runs at the tail
    dribble = [(24, c) for c in proj_chains(0, 0) + proj_chains(0, 1)]
    dribble += [(58, c) for c in proj_chains(1, 0)]
    emit_attention_batch(1, dribble)
    for _, chain in dribble:
        chain()
    emit_a2a(1, 1)
    for chain in proj_chains(1, 1):
        chain()
    ctx.close()


def _host_prep(x, w1, w2):
    x = np.ascontiguousarray(np.asarray(x, dtype=np.float32))
    w1 = np.ascontiguousarray(np.asarray(w1, dtype=np.float32))
    w2 = np.ascontiguousarray(np.asarray(w2, dtype=np.float32))

    xT = np.ascontiguousarray(x.reshape(R, E).T)          # [E, R]
    w2T = np.ascontiguousarray(w2.T)                      # [E, E]

    theta = 1.0 / (BASE ** (np.arange(0, HD, 2, dtype=np.float32) / HD))
    enc = np.arange(S, dtype=np.float32)[:, None] * theta[None, :]
    enc = np.repeat(enc, 2, axis=-1)                      # [s, 64]
    cos1 = np.cos(enc).T.astype(np.float32)               # [64, S]
    sin1 = np.sin(enc).T.astype(np.float32)
    cosT = np.ascontiguousarray(np.concatenate([cos1, cos1], axis=0))
    sinT = np.ascontiguousarray(np.concatenate([sin1, sin1], axis=0))

    m64 = np.zeros((HD, HD), dtype=np.float32)
    for i in range(HD // 2):
        m64[2 * i, 2 * i + 1] = -1.0
        m64[2 * i + 1, 2 * i] = 1.0
    m128 = np.zeros((128, 128), dtype=np.float32)
    m128[:64, :64] = m64
    m128[64:, 64:] = m64
    p2T = np.ascontiguousarray(m128.T)

    in_maps = []
    for c in range(N_CORES):
        hA, hB = HPC * c, HPC * c + 1
        def rows(base):
            return np.concatenate(
                [w1[base + hA * HD: base + (hA + 1) * HD, :],
                 w1[base + hB * HD: base + (hB + 1) * HD, :]], axis=0)
        in_maps.append({
            "xT": xT,
            "wqT": np.ascontiguousarray(rows(0).T),
            "wkT": np.ascontiguousarray(rows(E).T),
            "wvT": np.ascontiguousarray(rows(2 * E).T),
            "w2T": w2T,
            "cosT": cosT,
            "sinT": sinT,
            "p2T": p2T,
        })
    return in_maps


def kernel(x, w1, w2, _trace=False):
    if "nc" not in _COMPILED:
        _COMPILED["nc"] = _build_nc()
    nc = _COMPILED["nc"]
    in_maps = _host_prep(x, w1, w2)
    res = run_bass_kernel_spmd(nc, in_maps, core_ids=list(range(N_CORES)),
                               trace=_trace)
    _COMPILED["last_result"] = res
    # core c returns [512, E] as four 128-row blocks:
    # [b0 s=128c.., b0 s=1024+128c.., b1 s=128c.., b1 s=1024+128c..]
    full = np.empty((B, S, E), dtype=np.float32)
    for c in range(N_CORES):
        blk = res.results[c]["out"]
        full[0, 128 * c:128 * (c + 1)] = blk[0:128]
        full[0, 1024 + 128 * c:1024 + 128 * (c + 1)] = blk[128:256]
        full[1, 128 * c:128 * (c + 1)] = blk[256:384]
        full[1, 1024 + 128 * c:1024 + 128 * (c + 1)] = blk[384:512]
    return full


# revision 43
# speedup vs baseline: 1.1402x; 1.0342x over previous
"""Trainium2 Bass kernel for nn_Attention_12000138625343.

Full multi-head attention layer (B=2, S=2048, E=1024, H=16, hd=64, interleaved
RoPE on q/k, non-causal softmax) run tensor-parallel over 8 NeuronCores:

  - heads sharded 2-per-core (w1 columns / qkv projection sharded),
  - x replicated, passed pre-transposed [E, B*S] so the contraction dim lands
    on SBUF partitions,
  - scores computed transposed [k, q]; the two heads' K=64 score matmuls are
    packed into disjoint PE row-groups (concurrent), one exp instruction
    covers both heads' [128, 1024] PSUM block,
  - attn@v accumulates rolling per k-chunk with a ones-column appended to v
    producing the softmax denominator; the divide runs entirely off the
    TensorEngine (DVE reciprocal + DRAM-bounce broadcast DMA + DVE multiply),
  - batch-1 qkv projection / batch-0 output projection matmul chains are
    dribbled into the attention k-chunk loop so the in-order PE stream never
    starves the exp pipeline for long,
  - four AllToAlls (one per batch-half, each gated by that half's last
    softmax divide) convert the head sharding of the attention output o^T
    into row sharding; all but the final 0.5 MB collective hide under
    remaining compute, and each w2-projection block runs as soon as its
    half has arrived,
  - each core owns 2 x 128 rows of each batch; host reassembles.

Matmuls run in float32r (TF32-like, ~1e-4 relative error, full PE rate at
free-dim >= 256).
"""

import math

import numpy as np

import concourse.bass as bass
import concourse.mybir as mybir
import concourse.tile as tile
from concourse import bacc
from concourse.bass_utils import run_bass_kernel_spmd

B, S, E, H = 2, 2048, 1024, 16
HD = E // H  # 64
BASE = 10000.0
N_CORES = 8
HPC = H // N_CORES       # heads per core = 2
R = B * S                # 4096 flattened rows
RT = 512                 # rows per r-tile
NEC = E // 128           # 8 e-chunks of 128
QT = 512                 # q columns per q-tile
N_QT = S // QT           # 4 q-tiles per batch
KC = 128                 # k rows per k-chunk
N_KC = S // KC           # 16 k-chunks per batch
RPB = S // N_CORES       # rows per core per batch = 256

F32 = mybir.dt.float32
F32R = mybir.dt.float32r
EXPF = mybir.ActivationFunctionType.Exp

_COMPILED = {}


def _build_nc():
    nc = bacc.Bacc("TRN2", target_bir_lowering=False, debug=False,
                   num_devices=N_CORES)

    xT = nc.dram_tensor("xT", [E, R], F32, kind="ExternalInput").ap()
    wqT = nc.dram_tensor("wqT", [E, 128], F32, kind="ExternalInput").ap()
    wkT = nc.dram_tensor("wkT", [E, 128], F32, kind="ExternalInput").ap()
    wvT = nc.dram_tensor("wvT", [E, 128], F32, kind="ExternalInput").ap()
    w2T = nc.dram_tensor("w2T", [E, E], F32, kind="ExternalInput").ap()
    cosT = nc.dram_tensor("cosT", [128, S], F32, kind="ExternalInput").ap()
    sinT = nc.dram_tensor("sinT", [128, S], F32, kind="ExternalInput").ap()
    p2T = nc.dram_tensor("p2T", [128, 128], F32, kind="ExternalInput").ap()
    out = nc.dram_tensor("out", [2 * RPB, E], F32, kind="ExternalOutput").ap()

    with tile.TileContext(nc) as tc:
        _emit(tc, nc, xT, wqT, wkT, wvT, w2T, cosT, sinT, p2T, out)
    nc.compile()
    return nc


def _emit(tc, nc, xT, wqT, wkT, wvT, w2T, cosT, sinT, p2T, out):
    import contextlib
    ctx = contextlib.ExitStack()
    consts = ctx.enter_context(tc.tile_pool(name="consts", bufs=1))
    xtp = ctx.enter_context(tc.tile_pool(name="xtp", bufs=2))
    qkp = ctx.enter_context(tc.tile_pool(name="qkp", bufs=1))
    rawp = ctx.enter_context(tc.tile_pool(name="rawp", bufs=2))
    tmpp = ctx.enter_context(tc.tile_pool(name="tmpp", bufs=2))
    vp = ctx.enter_context(tc.tile_pool(name="vp", bufs=1))
    pp = ctx.enter_context(tc.tile_pool(name="pp", bufs=7))
    smallp = ctx.enter_context(tc.tile_pool(name="smallp", bufs=2))
    dramp = ctx.enter_context(tc.tile_pool(name="dramp", bufs=1, space="DRAM"))
    # PSUM budget (8 banks): qkv-shared 2 + sps 2 x 2 + av 2 = 8
    ps_qkv = ctx.enter_context(tc.tile_pool(name="ps_qkv", bufs=2, space="PSUM"))
    ps_sps = ctx.enter_context(tc.tile_pool(name="ps_sps", bufs=2, space="PSUM"))
    ps_av = ctx.enter_context(tc.tile_pool(name="ps_av", bufs=2, space="PSUM"))

    # ---- batched constant loads (single DMA each); q weights + first x tile
    # first so the first matmul can start ~10us in ----
    wq_f = xtp.tile([128, NEC, 128], F32, tag="recv", bufs=2, name="wq_f")
    nc.sync.dma_start(out=wq_f[:], in_=wqT.rearrange("(c p) f -> p c f", p=128))
    wq_all = consts.tile([128, NEC, 128], F32R, tag="wq", name="wq_all")
    nc.scalar.copy(wq_all[:], wq_f[:])
    xt0 = xtp.tile([128, NEC, RT], F32R, tag="xt", name="xt_0")
    nc.gpsimd.dma_start(
        out=xt0[:], in_=xT.rearrange("(c p) r -> p c r", p=128)[:, :, 0:RT])
    wk_all = consts.tile([128, NEC, 128], F32R, tag="wk", name="wk_all")
    nc.gpsimd.dma_start(out=wk_all[:], in_=wkT.rearrange("(c p) f -> p c f", p=128))
    wv_all = consts.tile([128, NEC, 128], F32R, tag="wv", name="wv_all")
    nc.gpsimd.dma_start(out=wv_all[:], in_=wvT.rearrange("(c p) f -> p c f", p=128))
    p2_sb = consts.tile([128, 128], F32R, tag="p2", name="p2_sb")
    nc.gpsimd.dma_start(out=p2_sb[:], in_=p2T[:, :])

    cos_sb = consts.tile([128, S], F32, tag="cos", name="cos_sb")
    nc.sync.dma_start(out=cos_sb[:], in_=cosT[:, :])
    sin_sb = consts.tile([128, S], F32, tag="sin", name="sin_sb")
    nc.sync.dma_start(out=sin_sb[:], in_=sinT[:, :])
    ones_f32 = consts.tile([128, 64], F32, tag="ones32", name="ones_f32")
    nc.vector.memset(ones_f32[:], 1.0)
    ones_r = consts.tile([1, 64], F32R, tag="onesr", name="ones_r")
    nc.vector.tensor_copy(ones_r[:], ones_f32[0:1, 0:64])

    # A2A buffers, one pair per (batch, half): [8 chunks, 128 e-rows, 128 rows]
    # half 0 carries s in [128j, 128j+128) (ready after q-tile 1),
    # half 1 carries s in [1024+128j, ...) (ready after q-tile 3).
    send_d = {(b, hf): dramp.tile([N_CORES, 128, 128], F32, name=f"send{b}{hf}")
              for b in range(B) for hf in range(2)}
    recv_d = {(b, hf): dramp.tile([N_CORES, 128, 128], F32, name=f"recv{b}{hf}")
              for b in range(B) for hf in range(2)}

    qT_sb, kT_sb, v_sb = {}, {}, {}
    w2_sb = {}

    def emit_xt_load(rt):
        if rt == 0:
            return xt0
        t = xtp.tile([128, NEC, RT], F32R, tag="xt", name=f"xt_{rt}")
        nc.gpsimd.dma_start(
            out=t[:],
            in_=xT.rearrange("(c p) r -> p c r", p=128)[:, :, rt * RT:(rt + 1) * RT])
        return t

    def qkv_chains(rt, xt):
        """Return a list of closures, each emitting one matmul chain (+ its
        epilogue) for r-tile rt. Callers dribble these between attention
        steps to keep the in-order PE stream dense but never monolithic."""
        b, st = rt // N_QT, (rt % N_QT) * RT

        if b not in qT_sb:
            qT_sb[b] = qkp.tile([128, S], F32R, tag=f"qT{b}", name=f"qT{b}")
            kT_sb[b] = qkp.tile([128, S], F32R, tag=f"kT{b}", name=f"kT{b}")

        def qk_chain(kind, w_all, dst):
            state = {}
            def emit_a():
                acc = ps_qkv.tile([128, RT], F32, tag="qkv",
                                  name=f"{kind}acc{rt}")
                for ec in range(4):
                    nc.tensor.matmul(acc[:], w_all[:, ec, :], xt[:, ec, :],
                                     start=(ec == 0), stop=False)
                state["acc"] = acc
            def emit_b():
                acc = state.pop("acc")
                for ec in range(4, NEC):
                    nc.tensor.matmul(acc[:], w_all[:, ec, :], xt[:, ec, :],
                                     start=False, stop=(ec == NEC - 1))
                raw = rawp.tile([128, RT], F32R, tag="raw",
                                name=f"{kind}raw{rt}")
                nc.scalar.copy(raw[:], acc[:])
                rot = ps_qkv.tile([128, RT], F32, tag="qkv",
                                  name=f"{kind}rot{rt}")
                nc.tensor.matmul(rot[:], p2_sb[:], raw[:], start=True, stop=True)
                t1 = tmpp.tile([128, RT], F32, tag="ropet", name=f"{kind}t1_{rt}")
                nc.vector.tensor_mul(t1[:], raw[:].bitcast(F32),
                                     cos_sb[:, st:st + RT])
                t2 = tmpp.tile([128, RT], F32, tag="ropet", name=f"{kind}t2_{rt}")
                nc.vector.tensor_mul(t2[:], rot[:], sin_sb[:, st:st + RT])
                nc.vector.tensor_add(dst[:, st:st + RT], t1[:], t2[:])
            return [emit_a, emit_b]

        def v_chain(sub):
            def emit():
                vacc = ps_qkv.tile([128, 128], F32, tag="qkv",
                                   name=f"vacc{rt}_{sub}")
                for ec in range(NEC):
                    nc.tensor.matmul(vacc[:],
                                     xt[:, ec, sub * 128:(sub + 1) * 128],
                                     wv_all[:, ec, :],
                                     start=(ec == 0), stop=(ec == NEC - 1))
                kc = (rt % N_QT) * 4 + sub
                for h in range(HPC):
                    vt = vp.tile([128, 65], F32R, tag=f"v{b}{h}{kc}",
                                 name=f"v{b}{h}{kc}")
                    nc.vector.tensor_copy(vt[:, 0:64],
                                          vacc[:, h * 64:(h + 1) * 64])
                    nc.vector.tensor_copy(vt[:, 64:65], ones_f32[:, 0:1])
                    v_sb[(b, h, kc)] = vt
            return emit

        return qk_chain("q", wq_all, qT_sb[b]) + \
               qk_chain("k", wk_all, kT_sb[b]) + \
               [v_chain(sub) for sub in range(4)]

    def proj_chains(b, hf):
        """Output projection for my 128 rows of (batch b, half hf).
        The recv load is emitted lazily by the first chain so that building
        the chain list never precedes the collective's emission."""
        state0 = {}
        def get_recv():
            if "t" not in state0:
                t = xtp.tile([128, NEC, 128], F32R, tag="recv", bufs=2,
                             name=f"recv{b}{hf}")
                nc.gpsimd.dma_start(
                    out=t[:], in_=recv_d[(b, hf)].rearrange("c p r -> p c r"))
                state0["t"] = t
            return state0["t"]
        chains = []
        for rblk in [hf]:
            for ft in range(2):
                state = {}
                def emit_a(rblk=rblk, ft=ft, state=state):
                    recv_sb = get_recv()
                    # qkv psum tag: free during attention (projection is done)
                    ops = ps_qkv.tile([128, 512], F32, tag="qkv",
                                      name=f"ops{b}_{rblk}_{ft}")
                    for ec in range(4):
                        nc.tensor.matmul(
                            ops[:],
                            recv_sb[:, ec, :],
                            w2_sb[0][:, ec, ft * 512:(ft + 1) * 512],
                            start=(ec == 0), stop=False)
                    state["ops"] = ops
                def emit_b(rblk=rblk, ft=ft, state=state):
                    recv_sb = get_recv()
                    ops = state.pop("ops")
                    for ec in range(4, NEC):
                        nc.tensor.matmul(
                            ops[:],
                            recv_sb[:, ec, :],
                            w2_sb[0][:, ec, ft * 512:(ft + 1) * 512],
                            start=False, stop=(ec == NEC - 1))
                    ot = tmpp.tile([128, 512], F32, tag="ropet",
                                   name=f"ot{b}_{rblk}_{ft}")
                    nc.scalar.copy(ot[:], ops[:])
                    # out rows: [b0h0, b0h1, b1h0, b1h1] blocks of 128
                    ob = 2 * b + rblk
                    nc.sync.dma_start(
                        out=out[ob * 128:(ob + 1) * 128,
                                ft * 512:(ft + 1) * 512],
                        in_=ot[:])
                chains.append(emit_a)
                chains.append(emit_b)
        return chains

    def emit_divide(b, qt, avs):
        """Divide by the softmax denominator (row 64 of av) and stage into
        the A2A send buffer. PE-free: broadcast via a DRAM bounce DMA.
        All DMAs on HWDGE queues (sync/scalar) so the collective sitting on
        the gpsimd queue can never block them."""
        last = (b == B - 1 and qt == N_QT - 1)
        for h in range(HPC):
            # evict the accumulator to SBUF immediately: releases the PSUM
            # slot so the next q-tile's attn@v never waits on this divide
            oraw = smallp.tile([65, QT], F32, tag="oraw", name=f"oraw{b}{h}{qt}")
            nc.scalar.copy(oraw[:], avs[h][:])
            rcp = smallp.tile([1, QT], F32R, tag="rcp", name=f"rcp{b}{h}{qt}")
            with nc.allow_low_precision(reason="f32r reciprocal ~1e-4"):
                nc.vector.reciprocal(rcp[:], oraw[64:65, :])
            bcs = smallp.tile([64, QT], F32, tag="bcs", name=f"bcs{b}{h}{qt}")
            if last:
                # PE is idle at the kernel tail: broadcast via a K=1 matmul
                # instead of the DRAM-bounce DMA round trip
                bcq = ps_av.tile([64, QT], F32, tag="av", name=f"bcq{b}{h}{qt}")
                nc.tensor.matmul(bcq[:], ones_r[:], rcp[:], start=True,
                                 stop=True)
                nc.scalar.copy(bcs[:], bcq[:])
            else:
                rcp_d = dramp.tile([1, QT], F32, tag="rcpd", bufs=4,
                                   name=f"rcpd{b}{h}{qt}")
                nc.sync.dma_start(out=rcp_d[:], in_=rcp[:].bitcast(F32))
                bcast = bass.AP(tensor=rcp_d.tensor, offset=rcp_d.offset,
                                ap=[[0, 64]] + list(rcp_d.ap[1:]))
                nc.sync.dma_start(out=bcs[:], in_=bcast)
            odiv = smallp.tile([64, QT], F32, tag="odiv", name=f"odiv{b}{h}{qt}")
            nc.vector.tensor_mul(odiv[:], oraw[0:64, :], bcs[:])
            # q-tile qt covers s in [512qt, 512qt+512): half hf = qt // 2,
            # destination cores j = 4*(qt%2) .. +4, 128 columns each
            hf = qt // 2
            for jj in range(4):
                j = 4 * (qt % 2) + jj
                nc.sync.dma_start(
                    out=send_d[(b, hf)][j, h * 64:(h + 1) * 64, :],
                    in_=odiv[:, jj * 128:(jj + 1) * 128])

    def emit_attention_batch(b, dribble):
        """All 4 q-tiles of a batch as one rolling pipeline over 64+LAG
        (qt, kc) units: scores+exp lead, attn@v trails by LAG units, the
        divide chain fires as each q-tile's accumulation completes.  One
        dribble chain (qkv projection / output projection) is popped every
        other unit to keep the in-order PE stream dense."""
        scale = 1.0 / math.sqrt(HD)
        NU = N_QT * N_KC
        LAG = 5
        pts = {}
        avs = {}
        for u in range(NU + LAG):
            if u < NU:
                qt, kc = divmod(u, N_KC)
                if kc == 0:
                    avs[qt] = [ps_av.tile([65, QT], F32, tag="av",
                                          name=f"av{b}{h}{qt}")
                               for h in range(HPC)]
                sps = ps_sps.tile([128, 2 * QT], F32, tag="sps",
                                  name=f"s{b}{qt}_{kc}")
                for h in range(HPC):
                    hof = h * 64
                    nc.tensor.matmul(
                        sps[:, h * QT:(h + 1) * QT],
                        kT_sb[b][hof:hof + 64, kc * KC:(kc + 1) * KC],
                        qT_sb[b][hof:hof + 64, qt * QT:(qt + 1) * QT],
                        start=True, stop=True)
                pt = pp.tile([128, 2 * QT], F32R, tag="p", name=f"p{b}{qt}_{kc}")
                nc.scalar.activation(pt[:], sps[:], EXPF, scale=scale)
                pts[u] = pt
            if u >= LAG:
                j = u - LAG
                qt2, kc2 = divmod(j, N_KC)
                for h in range(HPC):
                    nc.tensor.matmul(avs[qt2][h][:], v_sb[(b, h, kc2)][:],
                                     pts[j][:, h * QT:(h + 1) * QT],
                                     start=(kc2 == 0), stop=(kc2 == N_KC - 1))
                del pts[j]
                if kc2 == N_KC - 1:
                    emit_divide(b, qt2, avs.pop(qt2))
                    if qt2 == 1:
                        emit_a2a(b, 0)
            # one chain per two units, ramping up near the end so no
            # backlog remains to run as a monolithic lump afterwards
            if dribble and dribble[0][0] <= u and (
                    u % 2 == 1 or 2 * len(dribble) >= (NU + LAG - u)):
                dribble.pop(0)[1]()

    def emit_a2a(b, hf):
        nc.gpsimd.collective_compute(
            "AllToAll", mybir.AluOpType.bypass,
            replica_groups=[list(range(N_CORES))],
            ins=[send_d[(b, hf)].opt()], outs=[recv_d[(b, hf)].opt()])

    # ---------------- emission ----------------
    for rt in range(N_QT):             # batch-0 projection: pure PE stretch
        xt = emit_xt_load(rt)
        for chain in qkv_chains(rt, xt):
            chain()
    # warm the collective path (cold-start ~8us); emitted here so the wait on
    # the gpsimd queue never delays the critical first x/weight loads
    cwu_s = dramp.tile([N_CORES, 8], F32, tag="cwus", name="cwu_s")
    cwu_r = dramp.tile([N_CORES, 8], F32, tag="cwur", name="cwu_r")
    nc.sync.dma_start(out=cwu_s.rearrange("c r -> (c r)")[None, :],
                      in_=ones_f32[0:1, 0:64])
    nc.gpsimd.collective_compute(
        "AllToAll", mybir.AluOpType.bypass,
        replica_groups=[list(range(N_CORES))],
        ins=[cwu_s.opt()], outs=[cwu_r.opt()])

    # w2 load early: 4 MB, overlaps the batch-0 attention stretch
    w2_sb[0] = consts.tile([128, NEC, E], F32R, tag="w2", name="w2_all")
    nc.gpsimd.dma_start(out=w2_sb[0][:],
                        in_=w2T.rearrange("(c p) f -> p c f", p=128))
    # batch-0 attention with batch-1 qkv dribbled in
    dribble = []
    for qt in range(N_QT):
        rt = N_QT + qt
        xt = emit_xt_load(rt)
        dribble.extend(qkv_chains(rt, xt))
    dribble = [(1, c) for c in dribble]
    emit_attention_batch(0, dribble)
    for _, chain in dribble:
        chain()
    del dribble[:]
    emit_a2a(0, 1)                     # second half, fires at batch-0 end

    # both batch-0 halves complete early in batch-1 attention;
    # batch-1 half 0's A2A fires mid-batch, its projection runs at the tail
    dribble = [(24, c) for c in proj_chains(0, 0) + proj_chains(0, 1)]
    dribble += [(58, c) for c in proj_chains(1, 0)]
    emit_attention_batch(1, dribble)
    for _, chain in dribble:
        chain()
    emit_a2a(1, 1)
    for chain in proj_chains(1, 1):
        chain()
    ctx.close()


def _host_prep(x, w1, w2):
    x = np.ascontiguousarray(np.asarray(x, dtype=np.float32))
    w1 = np.ascontiguousarray(np.asarray(w1, dtype=np.float32))
    w2 = np.ascontiguousarray(np.asarray(w2, dtype=np.float32))

    xT = np.ascontiguousarray(x.reshape(R, E).T)          # [E, R]
    w2T = np.ascontiguousarray(w2.T)                      # [E, E]

    theta = 1.0 / (BASE ** (np.arange(0, HD, 2, dtype=np.float32) / HD))
    enc = np.arange(S, dtype=np.float32)[:, None] * theta[None, :]
    enc = np.repeat(enc, 2, axis=-1)                      # [s, 64]
    cos1 = np.cos(enc).T.astype(np.float32)               # [64, S]
    sin1 = np.sin(enc).T.astype(np.float32)
    cosT = np.ascontiguousarray(np.concatenate([cos1, cos1], axis=0))
    sinT = np.ascontiguousarray(np.concatenate([sin1, sin1], axis=0))

    m64 = np.zeros((HD, HD), dtype=np.float32)
    for i in range(HD // 2):
        m64[2 * i, 2 * i + 1] = -1.0
        m64[2 * i + 1, 2 * i] = 1.0
    m128 = np.zeros((128, 128), dtype=np.float32)
    m128[:64, :64] = m64
    m128[64:, 64:] = m64
    p2T = np.ascontiguousarray(m128.T)

    in_maps = []
    for c in range(N_CORES):
        hA, hB = HPC * c, HPC * c + 1
        def rows(base):
            return np.concatenate(
                [w1[base + hA * HD: base + (hA + 1) * HD, :],
                 w1[base + hB * HD: base + (hB + 1) * HD, :]], axis=0)
        in_maps.append({
            "xT": xT,
            "wqT": np.ascontiguousarray(rows(0).T),
            "wkT": np.ascontiguousarray(rows(E).T),
            "wvT": np.ascontiguousarray(rows(2 * E).T),
            "w2T": w2T,
            "cosT": cosT,
            "sinT": sinT,
            "p2T": p2T,
        })
    return in_maps


def kernel(x, w1, w2, _trace=False):
    if "nc" not in _COMPILED:
        _COMPILED["nc"] = _build_nc()
    nc = _COMPILED["nc"]
    in_maps = _host_prep(x, w1, w2)
    res = run_bass_kernel_spmd(nc, in_maps, core_ids=list(range(N_CORES)),
                               trace=_trace)
    _COMPILED["last_result"] = res
    # core c returns [512, E] as four 128-row blocks:
    # [b0 s=128c.., b0 s=1024+128c.., b1 s=128c.., b1 s=1024+128c..]
    full = np.empty((B, S, E), dtype=np.float32)
    for c in range(N_CORES):
        blk = res.results[c]["out"]
        full[0, 128 * c:128 * (c + 1)] = blk[0:128]
        full[0, 1024 + 128 * c:1024 + 128 * (c + 1)] = blk[128:256]
        full[1, 128 * c:128 * (c + 1)] = blk[256:384]
        full[1, 1024 + 128 * c:1024 + 128 * (c + 1)] = blk[384:512]
    return full
